# revision 54
# baseline (speedup 1.0000x reference)
"""Dilated attention kernel for Trainium2 (8 NeuronCores, SPMD).

Problem: B=4, H=8, L=2048, D=128, dilation ratios [1,2,4,8].
Inputs  query/key/value: [32, 2048, 128] f32 (grouped (b h)).
Output: [4, 2048, 1024] f32 (b, l, h*d).

Math: for ratio dr, head h attends within the strided position subset
{p : p % dr == r}, r = h >> (3 - log2 dr); results are scatter-added over
ratios.

Key structure: permute positions by sigma(p) = rev3(p%8)*256 + p//8. Under
sigma every (dr, r) position subset becomes a CONTIGUOUS 128-row-chunk
range, and for a fixed head the dr>1 score matrices are SUBMATRICES of the
dr=1 (full, dense) score matrix. So per head we compute the dense scores
S = K^T Q and E = exp(S - 20) exactly ONCE, and every ratio's attention is
E-submatrix @ V-submatrix plus its own row-sum normalizer:

  - dr=1 uses the full E (all 16 key chunks x the core's 8 query chunks)
  - dr in {2,4,8} uses E restricted to an aligned contiguous chunk block

The PV accumulations are organized so no (l, m) product is computed twice:
each query chunk's accumulation over the 16 key chunks is split into
segments at every applicable block boundary (the block family is laminar),
and every ratio's output is a chain of vector adds over segments that
reuses smaller outputs as partial sums (see _lc_plan).

Outputs ship UNNORMALIZED with their row-sum Z as a 129th column (the ones
column of the V operand yields Z for free inside the same matmul group);
the host divides. This removes all reciprocal/normalize work on-device.

Sharding: core c = (batch b=c//2, query-half qh=c%2). SPMD: all cores run
one identical program over 8 "slots". The host maps (head, query-half) data
into slots with a per-slot XOR relabeling of 128-row chunks (XOR maps
aligned power-of-two blocks to aligned blocks), which normalizes every
core's block layout to one static slot structure:

  slot 0: dr2@[0,8) dr4@[0,4) dr8@[0,2)     slot 4: dr4@[4,8) dr8@[2,4)
  slot 1: dr2@[0,8) dr4@[0,4)               slot 5: dr4@[4,8)
  slot 2: dr2@[0,8) dr8@[4,6)               slot 6: dr8@[6,8)
  slot 3: dr2@[0,8)                         slot 7: (dr1 only)

(program chunk c holds sigma chunk c ^ mask, mask = 8*qh ^ w(qh, slot);
the program's query chunks [0,8) are the core's own query half, and every
present block's key range lies in [0,8).)

On device, per slot: S^T = K Q^T in float32r (PE pseudo-fp32, 1 cyc/row at
free >= 256), exp on ScalarE (PSUM -> bf16 P^T tiles), PV groups in bf16
with the ones column appended to V pairs host-side (129-wide contiguous
rhs keeps DMA descriptors >= 512B).
"""

import numpy as np

B, H, L, D = 4, 8, 2048, 128
DRS = [1, 2, 4, 8]
REV3 = [0, 4, 2, 6, 1, 5, 3, 7]

# sigma and its inverse as row-index arrays
P_OF_PI = np.array([(pi % 256) * 8 + REV3[pi // 256] for pi in range(L)])
SIG = np.empty(L, np.int64)
SIG[P_OF_PI] = np.arange(L)

# static slot structure: (dr2 present, dr4 chunk range, dr8 chunk range)
SLOTS = [
    (True, (0, 4), (0, 2)),
    (True, (0, 4), None),
    (True, None, (4, 6)),
    (True, None, None),
    (False, (4, 8), (2, 4)),
    (False, (4, 8), None),
    (False, None, (6, 8)),
    (False, None, None),
]
# per (qh, slot): (head, w) with chunk mask = 8*qh ^ w
SLOT_HEAD = {
    0: [(s, 0) for s in range(8)],
    1: list(zip([7, 6, 5, 4, 3, 2, 1, 0], [6, 4, 6, 0, 6, 4, 6, 0])),
}
# ostage chunk layout per slot: [0:8]=dr1, [8:16]=dr2, [16:20]=dr4, [20:22]=dr8
OS_CHUNKS = 22


def _lc_plan(s, lc):
    """PV plan for slot s, query chunk lc.

    Returns (segments, outputs): segments is a list of (m0, m1) PSUM
    accumulation groups partitioning [0, 16) at every applicable block
    boundary; outputs maps ostage chunk -> list of segment indices to sum
    (every m-chunk is matmul'd exactly once; combining is vector work).
    """
    dr2p, r4, r8 = SLOTS[s]
    bounds = {0, 8, 16} if dr2p else {0, 16}
    if r4 is not None and r4[0] <= lc < r4[1]:
        bounds |= set(r4)
    if r8 is not None and r8[0] <= lc < r8[1]:
        bounds |= set(r8)
    cuts = sorted(bounds)
    segments = list(zip(cuts[:-1], cuts[1:]))

    def covering(a, b):
        return [i for i, (x, y) in enumerate(segments) if a <= x and y <= b]

    outputs = {lc: covering(0, 16)}                       # dr1
    if dr2p:
        outputs[8 + lc] = covering(0, 8)                  # dr2
    if r4 is not None and r4[0] <= lc < r4[1]:
        outputs[16 + (lc - r4[0])] = covering(*r4)        # dr4
    if r8 is not None and r8[0] <= lc < r8[1]:
        outputs[20 + (lc - r8[0])] = covering(*r8)        # dr8
    return segments, outputs


_CACHE = {}

CFG = {
    "lookahead": 2,   # S-phases emitted ahead of each PV
    "pt_bufs": 3,
    "ps_s_bufs": 3,
    "ps_o_bufs": 2,
    "work_bufs": 2,
    "store_eng": "sync",
}

# exp engine split: ACT does chunks [0,10) exactly; DVE handles [10,16)
# with a Schraudolph-style exp approximation. Q is pre-scaled by A7 =
# 128/ln2 on the host so the matmul emits A7*s directly; then
# i16 = max(A7*s + B7, 0) truncated to int16, bits viewed as bf16, is
# exp(s-20) with ~2-3% relative error (exactly +0.0 on underflow via the
# max). One fused DVE tensor_scalar per chunk group. The approximated E
# columns only feed the dr=1 output term (dr>1 blocks all live in chunks
# [0,8)); measured end-to-end absmax rel err 5.4e-3. The exact-exp ACT
# path compensates the A7 scaling with the activation's scale parameter.
EXP_A7 = 128.0 / np.log(2.0)
EXP_B7 = 127.0 * 128.0 - 0.05798 * 128.0 + 0.5 - 20.0 * EXP_A7
# S-phase consumer groups: (kind, chunk0, n); kind A=ACT exact exp,
# D=DVE approx. Order feeds both consumer engines early.
S_GROUPS = [
    ("A", 0, 2), ("D", 10, 2), ("A", 2, 2), ("D", 12, 2),
    ("A", 4, 2), ("D", 14, 2), ("A", 6, 2), ("A", 8, 2),
]


def _build():
    """Build + compile the SPMD Bass program (identical on all 8 cores)."""
    import concourse.bass as bass  # noqa: F401
    import concourse.mybir as mybir
    import concourse.tile as tile
    from concourse import bacc

    f32 = mybir.dt.float32
    f32r = mybir.dt.float32r
    bf16 = mybir.dt.bfloat16

    nc = bacc.Bacc()
    qt = nc.dram_tensor("qt", [8, D, 1024], f32r, kind="ExternalInput")
    kt = nc.dram_tensor("kt", [8, D, L], f32r, kind="ExternalInput")
    v2 = nc.dram_tensor("v2", [8, 128, 2064], bf16, kind="ExternalInput")
    o = nc.dram_tensor("o", [8, OS_CHUNKS * 128, 129], f32, kind="ExternalOutput")

    with tile.TileContext(nc) as tc:
        with (
            tc.tile_pool(name="singles", bufs=1) as singles,
            tc.tile_pool(name="work", bufs=CFG["work_bufs"]) as work,
            tc.tile_pool(name="pt_pool", bufs=CFG["pt_bufs"]) as pt_pool,
            tc.tile_pool(name="ps_s", bufs=CFG["ps_s_bufs"], space="PSUM") as ps_s,
            tc.tile_pool(name="ps_o", bufs=CFG["ps_o_bufs"], space="PSUM") as ps_o,
        ):
            # constant bias for exp(s - 20): keeps exp values in range without
            # a data-dependent row max (|s| <= ~70)
            exp_bias = singles.tile([128, 1], f32)
            nc.vector.memset(exp_bias, -20.0)

            # PE p-state warmup: the tensor engine reaches full clock only
            # after ~3us of continuous execution. The first real matmul waits
            # ~4.3us for the first DMAs, so burn that window with dummy
            # matmuls on a zeroed tile; the ramp then completes in the DMA
            # shadow and real matmuls start at full speed.
            warm = singles.tile([128, 512], bf16, name="warm")
            nc.vector.memset(warm, 0.0)
            for _ in range(CFG.get("warmup", 6)):
                psW = ps_s.tile([128, 2, 512], f32, tag="psS", name="psW")
                nc.tensor.matmul(
                    psW[:, 0, :],
                    lhsT=warm[:, 0:128],
                    rhs=warm,
                    start=True,
                    stop=True,
                )

            head_loads = []
            all_tasks = []
            for s in range(8):
                KT = work.tile([128, 16, 128], f32r, tag="KT")
                QT = work.tile([128, 8, 128], f32r, tag="QT")
                V2 = work.tile([128, 8, 258], bf16, tag="V2")
                ostage = work.tile([128, OS_CHUNKS, 129], f32, tag="ostage")

                def load(s=s, KT=KT, QT=QT, V2=V2):
                    def dk(a, b, eng=nc.sync):
                        eng.dma_start(
                            out=KT[:, a:b, :].rearrange("d c l -> d (c l)"),
                            in_=kt[s, :, a * 128 : b * 128],
                        )

                    def dq(a, b):
                        nc.sync.dma_start(
                            out=QT[:, a:b, :].rearrange("d c l -> d (c l)"),
                            in_=qt[s, :, a * 128 : b * 128],
                        )

                    # kt pieces in S_GROUPS consumption order: chunks
                    # [0,1] [10,11] [2,3] [12,13] [4,5] [14,15] [6,7] [8,9]
                    dq(0, 4)
                    dk(0, 2)
                    dk(2, 4)
                    dk(10, 14)
                    dk(4, 8)
                    dk(14, 16)
                    dk(8, 10)
                    dq(4, 8)
                    nc.sync.dma_start(
                        out=V2.rearrange("p c x -> p (c x)"), in_=v2[s]
                    )

                head_loads.append(load)

                def make_task(s, strip, PT, KT=KT, QT=QT, V2=V2, ostage=ostage):
                    def s_phase():
                        # S^T chunk matmuls (A7-prescaled q) + exp, 512 q
                        # columns. ACT groups: exact exp with scale=1/A7.
                        # DVE groups: fused (x + B7) max 0 -> int16 viewed
                        # as bf16 (Schraudolph).
                        for kind, mc0, npair in S_GROUPS:
                            psS = ps_s.tile([128, 2, 512], f32, tag="psS", name="psS")
                            for i in range(npair):
                                nc.tensor.matmul(
                                    psS[:, i, :],
                                    lhsT=KT[:, mc0 + i, :],
                                    rhs=QT[:, strip * 4 : strip * 4 + 4, :],
                                    start=True,
                                    stop=True,
                                )
                            if kind == "A":
                                nc.scalar.activation(
                                    out=PT[:, mc0 : mc0 + npair, :],
                                    in_=psS[:, 0:npair, :],
                                    func=mybir.ActivationFunctionType.Exp,
                                    bias=exp_bias,
                                    scale=1.0 / EXP_A7,
                                )
                            else:
                                nc.vector.tensor_scalar(
                                    out=PT[
                                        :, mc0 : mc0 + npair, :
                                    ].bitcast(mybir.dt.int16),
                                    in0=psS[:, 0:npair, :],
                                    scalar1=EXP_B7,
                                    scalar2=0.0,
                                    op0=mybir.AluOpType.add,
                                    op1=mybir.AluOpType.max,
                                )
                            yield

                    def pv_phase():
                        for lcl in range(4):
                            lc = strip * 4 + lcl
                            segments, outputs = _lc_plan(s, lc)
                            tiles = [
                                ps_o.tile([128, 3, 129], f32, tag="psO", name="psO")
                                for _ in range((len(segments) + 2) // 3)
                            ]
                            aps = [
                                tiles[g // 3][:, g % 3, :]
                                for g in range(len(segments))
                            ]
                            for g, (m0, m1) in enumerate(segments):
                                for mc in range(m0, m1):
                                    nc.tensor.matmul(
                                        aps[g],
                                        lhsT=PT[:, mc, lcl * 128 : (lcl + 1) * 128],
                                        rhs=V2[
                                            :,
                                            mc // 2,
                                            (mc % 2) * 129 : (mc % 2) * 129 + 129,
                                        ],
                                        start=(mc == m0),
                                        stop=(mc == m1 - 1),
                                    )
                            # combine segments into staged outputs, reusing
                            # smaller outputs as partial sums (blocks are
                            # laminar). Outputs processed smallest-first.
                            done = {}  # (m0, m1) range -> ostage chunk
                            for oc in sorted(
                                outputs, key=lambda c: len(outputs[c])
                            ):
                                segs = outputs[oc]
                                lo_, hi_ = (
                                    segments[segs[0]][0],
                                    segments[segs[-1]][1],
                                )
                                # greedy cover of [lo_, hi_): prefer computed
                                # sub-outputs, else raw segments
                                terms = []
                                pos = lo_
                                while pos < hi_:
                                    best = None
                                    for (a, b), c in done.items():
                                        if a == pos and b <= hi_ and (
                                            best is None or b > best[0]
                                        ):
                                            best = (b, ("chunk", c))
                                    if best is None:
                                        gi = next(
                                            i
                                            for i, (a, b) in enumerate(segments)
                                            if a == pos
                                        )
                                        best = (
                                            segments[gi][1],
                                            ("seg", gi),
                                        )
                                    pos = best[0]
                                    terms.append(best[1])
                                dst = ostage[:, oc, :]

                                def ap_of(term):
                                    kind, i = term
                                    return (
                                        aps[i]
                                        if kind == "seg"
                                        else ostage[:, i, :]
                                    )

                                if len(terms) == 1:
                                    nc.vector.tensor_copy(
                                        out=dst, in_=ap_of(terms[0])
                                    )
                                else:
                                    nc.vector.tensor_add(
                                        out=dst,
                                        in0=ap_of(terms[1]),
                                        in1=ap_of(terms[0]),
                                    )
                                    for term in terms[2:]:
                                        nc.vector.tensor_add(
                                            out=dst, in0=dst, in1=ap_of(term)
                                        )
                                done[(lo_, hi_)] = oc
                            yield
                        # store every ostage run whose source l-chunks are
                        # complete after this strip; the last slot splits its
                        # final run so the kernel tail ends on a small DMA
                        store_eng = getattr(nc, CFG["store_eng"])
                        dr2p, r4, r8 = SLOTS[s]
                        lo, hi = strip * 4, strip * 4 + 4
                        runs = []
                        runs.append((lo, hi))                       # dr1 part
                        if dr2p:
                            runs.append((8 + lo, 8 + hi))           # dr2 part
                        if r4 is not None and r4 == (lo, hi):
                            runs.append((16, 20))
                        if r8 is not None and lo <= r8[0] < hi:
                            runs.append((20, 22))
                        if s == 7 and strip == 1:
                            # keep the final DMA tiny: it gates kernel end
                            merged = [(4, 7), (7, 8)]
                        else:
                            runs.sort()
                            merged = [list(runs[0])]
                            for a, b in runs[1:]:
                                if a == merged[-1][1]:
                                    merged[-1][1] = b
                                else:
                                    merged.append([a, b])
                        for a, b in merged:
                            store_eng.dma_start(
                                out=o[s, a * 128 : b * 128, :].rearrange(
                                    "(c p) d -> p c d", p=128
                                ),
                                in_=ostage[:, a:b, :],
                            )

                    return s_phase, pv_phase

                for strip in range(2):
                    PT = pt_pool.tile([128, 16, 512], bf16, tag="pt", name="PT")
                    all_tasks.append(make_task(s, strip, PT))

            # software pipeline: emit S(i+LOOK) before PV(i); loads one slot
            # ahead so HWDGE ring order matches consumption order
            emitted_loads = [False] * 8

            def ensure_loads(j):
                if 0 <= j < 8 and not emitted_loads[j]:
                    emitted_loads[j] = True
                    head_loads[j]()

            def drain(gen):
                for _ in gen:
                    pass

            LOOK = CFG["lookahead"]
            NT = len(all_tasks)
            ensure_loads(0)
            ensure_loads(1)
            if CFG.get("ilv"):
                # fine-grained interleave: R S-groups emitted per PV yield,
                # S-stream runs up to LOOK tasks ahead of the PV stream
                R = CFG.get("ilv_ratio", 3)
                s_gens = [t[0]() for t in all_tasks]
                s_done = [False] * NT
                s_next = 0

                def step_s(limit, n):
                    nonlocal s_next
                    took = 0
                    while took < n and s_next <= min(limit, NT - 1):
                        if s_done[s_next]:
                            s_next += 1
                            continue
                        ensure_loads(s_next // 2 + 1)
                        try:
                            next(s_gens[s_next])
                            took += 1
                        except StopIteration:
                            s_done[s_next] = True
                            s_next += 1

                for i in range(NT):
                    step_s(i, 10**9)
                    while not s_done[i]:
                        step_s(i, 10**9)
                    for _ in all_tasks[i][1]():
                        step_s(i + LOOK, R)
            else:
                for j in range(min(LOOK, NT)):
                    drain(all_tasks[j][0]())
                for i in range(NT):
                    # PV first: its DVE combines queue ahead of the next S
                    # phase's DVE exp ops, so psO buffers recycle promptly
                    drain(all_tasks[i][1]())
                    if i + LOOK < NT:
                        ensure_loads((i + LOOK) // 2 + 1)
                        drain(all_tasks[i + LOOK][0]())

    nc.compile()
    return nc


def _get_nc():
    if "nc" not in _CACHE:
        _CACHE["nc"] = _build()
    return _CACHE["nc"]


def _chunk_rows(mask):
    c = np.arange(16) ^ mask
    return (c[:, None] * 128 + np.arange(128)[None, :]).reshape(-1)


def _make_in_maps(query, key, value):
    import ml_dtypes

    q = query.reshape(B, H, L, D)[:, :, P_OF_PI, :]
    k = key.reshape(B, H, L, D)[:, :, P_OF_PI, :]
    v = value.reshape(B, H, L, D)[:, :, P_OF_PI, :]
    in_maps = []
    for c in range(8):
        b, qh = c // 2, c % 2
        ktd = np.empty((8, D, L), np.float32)
        qtd = np.empty((8, D, 1024), np.float32)
        v2d = np.ones((8, 128, 8, 2, 129), np.float32)
        for s in range(8):
            h, w = SLOT_HEAD[qh][s]
            rows = _chunk_rows(8 * qh ^ w)
            ktd[s] = k[b, h][rows].T
            # A7 pre-scale: the S matmul emits A7*s (see EXP_A7 notes)
            qtd[s] = q[b, h][rows[:1024]].T * EXP_A7
            # V pairs: [p, t, half, 0:128] = v row 256t + 128 half + p; col 128 = 1
            v2d[s, :, :, :, 0:128] = (
                v[b, h][rows].reshape(8, 2, 128, 128).transpose(2, 0, 1, 3)
            )
        in_maps.append(
            {
                "qt": qtd,
                "kt": ktd,
                "v2": np.ascontiguousarray(
                    v2d.reshape(8, 128, 2064)
                ).astype(ml_dtypes.bfloat16),
            }
        )
    return in_maps


def _assemble(results):
    total_sig = np.zeros((B, H, L, D), np.float32)
    for c in range(8):
        b, qh = c // 2, c % 2
        oc = np.asarray(results[c]["o"], np.float32)  # [8, 22*128, 129]
        for s in range(8):
            h, w = SLOT_HEAD[qh][s]
            rows = _chunk_rows(8 * qh ^ w)
            dr2p, r4, r8 = SLOTS[s]
            sections = [(0, 0, 1024)]  # (ostage chunk, prog row0, nrows)
            if dr2p:
                sections.append((8, 0, 1024))
            if r4 is not None:
                sections.append((16, r4[0] * 128, 512))
            if r8 is not None:
                sections.append((20, r8[0] * 128, 256))
            for oc0, pr0, nr in sections:
                blk = oc[s, oc0 * 128 : oc0 * 128 + nr]
                total_sig[b, h, rows[pr0 : pr0 + nr]] += (
                    blk[:, 0:128] / blk[:, 128:129]
                )
    total = total_sig[:, :, SIG, :]
    return np.ascontiguousarray(
        total.transpose(0, 2, 1, 3).reshape(B, L, H * D)
    )


def _run(query, key, value, trace=False, **trace_kwargs):
    from concourse.bass_utils import run_bass_kernel_spmd

    nc = _get_nc()
    in_maps = _make_in_maps(query, key, value)
    res = run_bass_kernel_spmd(
        nc, in_maps, list(range(8)), trace=trace, **trace_kwargs
    )
    return _assemble(res.results), res


def kernel(query, key, value):
    # accept any array-like (np, jax, lists) and normalize to f32 numpy
    query = np.asarray(query, dtype=np.float32)
    key = np.asarray(key, dtype=np.float32)
    value = np.asarray(value, dtype=np.float32)

    # The axon-tunneled devices occasionally drop a dispatch with a
    # transient NRT_EXEC_UNIT_UNRECOVERABLE / mesh-desync error that a
    # fresh attempt survives; retry rather than failing the whole call.
    import time

    last_err = None
    for attempt in range(3):
        try:
            out, _ = _run(query, key, value)
            return out
        except Exception as e:  # noqa: BLE001 - deliberate broad retry
            last_err = e
            time.sleep(5 * (attempt + 1))
    raise last_err


# revision 57
# speedup vs baseline: 1.0065x; 1.0065x over previous
"""Dilated attention kernel for Trainium2 (8 NeuronCores, SPMD).

Problem: B=4, H=8, L=2048, D=128, dilation ratios [1,2,4,8].
Inputs  query/key/value: [32, 2048, 128] f32 (grouped (b h)).
Output: [4, 2048, 1024] f32 (b, l, h*d).

Math: for ratio dr, head h attends within the strided position subset
{p : p % dr == r}, r = h >> (3 - log2 dr); results are scatter-added over
ratios.

Key structure: permute positions by sigma(p) = rev3(p%8)*256 + p//8. Under
sigma every (dr, r) position subset becomes a CONTIGUOUS 128-row-chunk
range, and for a fixed head the dr>1 score matrices are SUBMATRICES of the
dr=1 (full, dense) score matrix. So per head we compute the dense scores
S = K^T Q and E = exp(S - 20) exactly ONCE, and every ratio's attention is
E-submatrix @ V-submatrix plus its own row-sum normalizer:

  - dr=1 uses the full E (all 16 key chunks x the core's 8 query chunks)
  - dr in {2,4,8} uses E restricted to an aligned contiguous chunk block

The PV accumulations are organized so no (l, m) product is computed twice:
each query chunk's accumulation over the 16 key chunks is split into
segments at every applicable block boundary (the block family is laminar),
and every ratio's output is a chain of vector adds over segments that
reuses smaller outputs as partial sums (see _lc_plan).

Outputs ship UNNORMALIZED with their row-sum Z as a 129th column (the ones
column of the V operand yields Z for free inside the same matmul group);
the host divides. This removes all reciprocal/normalize work on-device.

Sharding: core c = (batch b=c//2, query-half qh=c%2). SPMD: all cores run
one identical program over 8 "slots". The host maps (head, query-half) data
into slots with a per-slot XOR relabeling of 128-row chunks (XOR maps
aligned power-of-two blocks to aligned blocks), which normalizes every
core's block layout to one static slot structure:

  slot 0: dr2@[0,8) dr4@[0,4) dr8@[0,2)     slot 4: dr4@[4,8) dr8@[2,4)
  slot 1: dr2@[0,8) dr4@[0,4)               slot 5: dr4@[4,8)
  slot 2: dr2@[0,8) dr8@[4,6)               slot 6: dr8@[6,8)
  slot 3: dr2@[0,8)                         slot 7: (dr1 only)

(program chunk c holds sigma chunk c ^ mask, mask = 8*qh ^ w(qh, slot);
the program's query chunks [0,8) are the core's own query half, and every
present block's key range lies in [0,8).)

On device, per slot: S^T = K Q^T in float32r (PE pseudo-fp32, 1 cyc/row at
free >= 256), exp on ScalarE (PSUM -> bf16 P^T tiles), PV groups in bf16
with the ones column appended to V pairs host-side (129-wide contiguous
rhs keeps DMA descriptors >= 512B).
"""

import numpy as np

B, H, L, D = 4, 8, 2048, 128
DRS = [1, 2, 4, 8]
REV3 = [0, 4, 2, 6, 1, 5, 3, 7]

# sigma and its inverse as row-index arrays
P_OF_PI = np.array([(pi % 256) * 8 + REV3[pi // 256] for pi in range(L)])
SIG = np.empty(L, np.int64)
SIG[P_OF_PI] = np.arange(L)

# static slot structure: (dr2 present, dr4 chunk range, dr8 chunk range)
SLOTS = [
    (True, (0, 4), (0, 2)),
    (True, (0, 4), None),
    (True, None, (4, 6)),
    (True, None, None),
    (False, (4, 8), (2, 4)),
    (False, (4, 8), None),
    (False, None, (6, 8)),
    (False, None, None),
]
# per (qh, slot): (head, w) with chunk mask = 8*qh ^ w
SLOT_HEAD = {
    0: [(s, 0) for s in range(8)],
    1: list(zip([7, 6, 5, 4, 3, 2, 1, 0], [6, 4, 6, 0, 6, 4, 6, 0])),
}
# ostage chunk layout per slot: [0:8]=dr1, [8:16]=dr2, [16:20]=dr4, [20:22]=dr8
OS_CHUNKS = 22


def _lc_plan(s, lc):
    """PV plan for slot s, query chunk lc.

    Returns (segments, outputs): segments is a list of (m0, m1) PSUM
    accumulation groups partitioning [0, 16) at every applicable block
    boundary; outputs maps ostage chunk -> list of segment indices to sum
    (every m-chunk is matmul'd exactly once; combining is vector work).
    """
    dr2p, r4, r8 = SLOTS[s]
    bounds = {0, 8, 16} if dr2p else {0, 16}
    if r4 is not None and r4[0] <= lc < r4[1]:
        bounds |= set(r4)
    if r8 is not None and r8[0] <= lc < r8[1]:
        bounds |= set(r8)
    cuts = sorted(bounds)
    segments = list(zip(cuts[:-1], cuts[1:]))

    def covering(a, b):
        return [i for i, (x, y) in enumerate(segments) if a <= x and y <= b]

    outputs = {lc: covering(0, 16)}                       # dr1
    if dr2p:
        outputs[8 + lc] = covering(0, 8)                  # dr2
    if r4 is not None and r4[0] <= lc < r4[1]:
        outputs[16 + (lc - r4[0])] = covering(*r4)        # dr4
    if r8 is not None and r8[0] <= lc < r8[1]:
        outputs[20 + (lc - r8[0])] = covering(*r8)        # dr8
    return segments, outputs


_CACHE = {}

CFG = {
    "lookahead": 2,   # S-phases emitted ahead of each PV
    "pt_bufs": 3,
    "ps_s_bufs": 6,
    "ps_o_bufs": 2,
    "work_bufs": 2,
    "store_eng": "sync",
}

# exp engine split: ACT does chunks [0,10) exactly; DVE handles [10,16)
# with a Schraudolph-style exp approximation. Q is pre-scaled by A7 =
# 128/ln2 on the host so the matmul emits A7*s directly; then
# i16 = max(A7*s + B7, 0) truncated to int16, bits viewed as bf16, is
# exp(s-20) with ~2-3% relative error (exactly +0.0 on underflow via the
# max). One fused DVE tensor_scalar per chunk group. The approximated E
# columns only feed the dr=1 output term (dr>1 blocks all live in chunks
# [0,8)); measured end-to-end absmax rel err 5.4e-3. The exact-exp ACT
# path compensates the A7 scaling with the activation's scale parameter.
EXP_A7 = 128.0 / np.log(2.0)
EXP_B7 = 127.0 * 128.0 - 0.05798 * 128.0 + 0.5 - 20.0 * EXP_A7
# S-phase consumer groups: (kind, chunk0, n); kind A=ACT exact exp,
# D=DVE approx. Order feeds both consumer engines early.
S_GROUPS = [
    ("A", 0, 1), ("A", 1, 1), ("D", 10, 1), ("A", 2, 1), ("D", 11, 1),
    ("A", 3, 1), ("D", 12, 1), ("A", 4, 1), ("D", 13, 1), ("A", 5, 1),
    ("D", 14, 1), ("A", 6, 1), ("D", 15, 1), ("A", 7, 1), ("A", 8, 1),
    ("A", 9, 1),
]


def _build():
    """Build + compile the SPMD Bass program (identical on all 8 cores)."""
    import concourse.bass as bass  # noqa: F401
    import concourse.mybir as mybir
    import concourse.tile as tile
    from concourse import bacc

    f32 = mybir.dt.float32
    f32r = mybir.dt.float32r
    bf16 = mybir.dt.bfloat16

    nc = bacc.Bacc()
    qt = nc.dram_tensor("qt", [8, D, 1024], f32r, kind="ExternalInput")
    kt = nc.dram_tensor("kt", [8, D, L], f32r, kind="ExternalInput")
    v2 = nc.dram_tensor("v2", [8, 128, 2064], bf16, kind="ExternalInput")
    o = nc.dram_tensor("o", [8, OS_CHUNKS * 128, 129], f32, kind="ExternalOutput")

    with tile.TileContext(nc) as tc:
        with (
            tc.tile_pool(name="singles", bufs=1) as singles,
            tc.tile_pool(name="work", bufs=CFG["work_bufs"]) as work,
            tc.tile_pool(name="pt_pool", bufs=CFG["pt_bufs"]) as pt_pool,
            tc.tile_pool(name="ps_s", bufs=CFG["ps_s_bufs"], space="PSUM") as ps_s,
            tc.tile_pool(name="ps_o", bufs=CFG["ps_o_bufs"], space="PSUM") as ps_o,
        ):
            # constant bias for exp(s - 20): keeps exp values in range without
            # a data-dependent row max (|s| <= ~70)
            exp_bias = singles.tile([128, 1], f32)
            nc.vector.memset(exp_bias, -20.0)

            # PE p-state warmup: the tensor engine reaches full clock only
            # after ~3us of continuous execution. The first real matmul waits
            # ~4.3us for the first DMAs, so burn that window with dummy
            # matmuls on a zeroed tile; the ramp then completes in the DMA
            # shadow and real matmuls start at full speed.
            warm = singles.tile([128, 512], bf16, name="warm")
            nc.vector.memset(warm, 0.0)
            for _ in range(CFG.get("warmup", 6)):
                psW = ps_s.tile([128, 1, 512], f32, tag="psS", name="psW")
                nc.tensor.matmul(
                    psW[:, 0, :],
                    lhsT=warm[:, 0:128],
                    rhs=warm,
                    start=True,
                    stop=True,
                )

            head_loads = []
            all_tasks = []
            for s in range(8):
                KT = work.tile([128, 16, 128], f32r, tag="KT")
                QT = work.tile([128, 8, 128], f32r, tag="QT")
                V2 = work.tile([128, 8, 258], bf16, tag="V2")
                ostage = work.tile([128, OS_CHUNKS, 129], f32, tag="ostage")

                def load(s=s, KT=KT, QT=QT, V2=V2):
                    def dk(a, b, eng=nc.sync):
                        eng.dma_start(
                            out=KT[:, a:b, :].rearrange("d c l -> d (c l)"),
                            in_=kt[s, :, a * 128 : b * 128],
                        )

                    def dq(a, b):
                        nc.sync.dma_start(
                            out=QT[:, a:b, :].rearrange("d c l -> d (c l)"),
                            in_=qt[s, :, a * 128 : b * 128],
                        )

                    # kt pieces in S_GROUPS consumption order: chunks
                    # [0,1] [10,11] [2,3] [12,13] [4,5] [14,15] [6,7] [8,9]
                    dq(0, 4)
                    dk(0, 2)
                    dk(2, 4)
                    dk(10, 14)
                    dk(4, 8)
                    dk(14, 16)
                    dk(8, 10)
                    dq(4, 8)
                    nc.sync.dma_start(
                        out=V2.rearrange("p c x -> p (c x)"), in_=v2[s]
                    )

                head_loads.append(load)

                def make_task(s, strip, PT, KT=KT, QT=QT, V2=V2, ostage=ostage):
                    def s_phase():
                        # S^T chunk matmuls (A7-prescaled q) + exp, 512 q
                        # columns. ACT groups: exact exp with scale=1/A7.
                        # DVE groups: fused (x + B7) max 0 -> int16 viewed
                        # as bf16 (Schraudolph).
                        for kind, mc0, npair in S_GROUPS:
                            psS = ps_s.tile([128, 1, 512], f32, tag="psS", name="psS")
                            for i in range(npair):
                                nc.tensor.matmul(
                                    psS[:, i, :],
                                    lhsT=KT[:, mc0 + i, :],
                                    rhs=QT[:, strip * 4 : strip * 4 + 4, :],
                                    start=True,
                                    stop=True,
                                )
                            if kind == "A":
                                nc.scalar.activation(
                                    out=PT[:, mc0 : mc0 + npair, :],
                                    in_=psS[:, 0:npair, :],
                                    func=mybir.ActivationFunctionType.Exp,
                                    bias=exp_bias,
                                    scale=1.0 / EXP_A7,
                                )
                            else:
                                nc.vector.tensor_scalar(
                                    out=PT[
                                        :, mc0 : mc0 + npair, :
                                    ].bitcast(mybir.dt.int16),
                                    in0=psS[:, 0:npair, :],
                                    scalar1=EXP_B7,
                                    scalar2=0.0,
                                    op0=mybir.AluOpType.add,
                                    op1=mybir.AluOpType.max,
                                )
                            yield

                    def pv_phase():
                        for lcl in range(4):
                            lc = strip * 4 + lcl
                            segments, outputs = _lc_plan(s, lc)
                            tiles = [
                                ps_o.tile([128, 3, 129], f32, tag="psO", name="psO")
                                for _ in range((len(segments) + 2) // 3)
                            ]
                            aps = [
                                tiles[g // 3][:, g % 3, :]
                                for g in range(len(segments))
                            ]
                            for g, (m0, m1) in enumerate(segments):
                                for mc in range(m0, m1):
                                    nc.tensor.matmul(
                                        aps[g],
                                        lhsT=PT[:, mc, lcl * 128 : (lcl + 1) * 128],
                                        rhs=V2[
                                            :,
                                            mc // 2,
                                            (mc % 2) * 129 : (mc % 2) * 129 + 129,
                                        ],
                                        start=(mc == m0),
                                        stop=(mc == m1 - 1),
                                    )
                            # combine segments into staged outputs, reusing
                            # smaller outputs as partial sums (blocks are
                            # laminar). Outputs processed smallest-first.
                            done = {}  # (m0, m1) range -> ostage chunk
                            for oc in sorted(
                                outputs, key=lambda c: len(outputs[c])
                            ):
                                segs = outputs[oc]
                                lo_, hi_ = (
                                    segments[segs[0]][0],
                                    segments[segs[-1]][1],
                                )
                                # greedy cover of [lo_, hi_): prefer computed
                                # sub-outputs, else raw segments
                                terms = []
                                pos = lo_
                                while pos < hi_:
                                    best = None
                                    for (a, b), c in done.items():
                                        if a == pos and b <= hi_ and (
                                            best is None or b > best[0]
                                        ):
                                            best = (b, ("chunk", c))
                                    if best is None:
                                        gi = next(
                                            i
                                            for i, (a, b) in enumerate(segments)
                                            if a == pos
                                        )
                                        best = (
                                            segments[gi][1],
                                            ("seg", gi),
                                        )
                                    pos = best[0]
                                    terms.append(best[1])
                                dst = ostage[:, oc, :]

                                def ap_of(term):
                                    kind, i = term
                                    return (
                                        aps[i]
                                        if kind == "seg"
                                        else ostage[:, i, :]
                                    )

                                if len(terms) == 1:
                                    nc.vector.tensor_copy(
                                        out=dst, in_=ap_of(terms[0])
                                    )
                                else:
                                    nc.vector.tensor_add(
                                        out=dst,
                                        in0=ap_of(terms[1]),
                                        in1=ap_of(terms[0]),
                                    )
                                    for term in terms[2:]:
                                        nc.vector.tensor_add(
                                            out=dst, in0=dst, in1=ap_of(term)
                                        )
                                done[(lo_, hi_)] = oc
                            yield
                        # store every ostage run whose source l-chunks are
                        # complete after this strip; the last slot splits its
                        # final run so the kernel tail ends on a small DMA
                        store_eng = getattr(nc, CFG["store_eng"])
                        dr2p, r4, r8 = SLOTS[s]
                        lo, hi = strip * 4, strip * 4 + 4
                        runs = []
                        runs.append((lo, hi))                       # dr1 part
                        if dr2p:
                            runs.append((8 + lo, 8 + hi))           # dr2 part
                        if r4 is not None and r4 == (lo, hi):
                            runs.append((16, 20))
                        if r8 is not None and lo <= r8[0] < hi:
                            runs.append((20, 22))
                        if s == 7 and strip == 1:
                            # keep the final DMA tiny: it gates kernel end
                            merged = [(4, 7), (7, 8)]
                        else:
                            runs.sort()
                            merged = [list(runs[0])]
                            for a, b in runs[1:]:
                                if a == merged[-1][1]:
                                    merged[-1][1] = b
                                else:
                                    merged.append([a, b])
                        for a, b in merged:
                            store_eng.dma_start(
                                out=o[s, a * 128 : b * 128, :].rearrange(
                                    "(c p) d -> p c d", p=128
                                ),
                                in_=ostage[:, a:b, :],
                            )

                    return s_phase, pv_phase

                for strip in range(2):
                    PT = pt_pool.tile([128, 16, 512], bf16, tag="pt", name="PT")
                    all_tasks.append(make_task(s, strip, PT))

            # software pipeline: emit S(i+LOOK) before PV(i); loads one slot
            # ahead so HWDGE ring order matches consumption order
            emitted_loads = [False] * 8

            def ensure_loads(j):
                if 0 <= j < 8 and not emitted_loads[j]:
                    emitted_loads[j] = True
                    head_loads[j]()

            def drain(gen):
                for _ in gen:
                    pass

            LOOK = CFG["lookahead"]
            NT = len(all_tasks)
            ensure_loads(0)
            ensure_loads(1)
            if CFG.get("ilv"):
                # fine-grained interleave: R S-groups emitted per PV yield,
                # S-stream runs up to LOOK tasks ahead of the PV stream
                R = CFG.get("ilv_ratio", 3)
                s_gens = [t[0]() for t in all_tasks]
                s_done = [False] * NT
                s_next = 0

                def step_s(limit, n):
                    nonlocal s_next
                    took = 0
                    while took < n and s_next <= min(limit, NT - 1):
                        if s_done[s_next]:
                            s_next += 1
                            continue
                        ensure_loads(s_next // 2 + 1)
                        try:
                            next(s_gens[s_next])
                            took += 1
                        except StopIteration:
                            s_done[s_next] = True
                            s_next += 1

                for i in range(NT):
                    step_s(i, 10**9)
                    while not s_done[i]:
                        step_s(i, 10**9)
                    for _ in all_tasks[i][1]():
                        step_s(i + LOOK, R)
            else:
                for j in range(min(LOOK, NT)):
                    drain(all_tasks[j][0]())
                for i in range(NT):
                    # PV first: its DVE combines queue ahead of the next S
                    # phase's DVE exp ops, so psO buffers recycle promptly
                    drain(all_tasks[i][1]())
                    if i + LOOK < NT:
                        ensure_loads((i + LOOK) // 2 + 1)
                        drain(all_tasks[i + LOOK][0]())

    nc.compile()
    return nc


def _get_nc():
    if "nc" not in _CACHE:
        _CACHE["nc"] = _build()
    return _CACHE["nc"]


def _chunk_rows(mask):
    c = np.arange(16) ^ mask
    return (c[:, None] * 128 + np.arange(128)[None, :]).reshape(-1)


def _make_in_maps(query, key, value):
    import ml_dtypes

    q = query.reshape(B, H, L, D)[:, :, P_OF_PI, :]
    k = key.reshape(B, H, L, D)[:, :, P_OF_PI, :]
    v = value.reshape(B, H, L, D)[:, :, P_OF_PI, :]
    in_maps = []
    for c in range(8):
        b, qh = c // 2, c % 2
        ktd = np.empty((8, D, L), np.float32)
        qtd = np.empty((8, D, 1024), np.float32)
        v2d = np.ones((8, 128, 8, 2, 129), np.float32)
        for s in range(8):
            h, w = SLOT_HEAD[qh][s]
            rows = _chunk_rows(8 * qh ^ w)
            ktd[s] = k[b, h][rows].T
            # A7 pre-scale: the S matmul emits A7*s (see EXP_A7 notes)
            qtd[s] = q[b, h][rows[:1024]].T * EXP_A7
            # V pairs: [p, t, half, 0:128] = v row 256t + 128 half + p; col 128 = 1
            v2d[s, :, :, :, 0:128] = (
                v[b, h][rows].reshape(8, 2, 128, 128).transpose(2, 0, 1, 3)
            )
        in_maps.append(
            {
                "qt": qtd,
                "kt": ktd,
                "v2": np.ascontiguousarray(
                    v2d.reshape(8, 128, 2064)
                ).astype(ml_dtypes.bfloat16),
            }
        )
    return in_maps


def _assemble(results):
    total_sig = np.zeros((B, H, L, D), np.float32)
    for c in range(8):
        b, qh = c // 2, c % 2
        oc = np.asarray(results[c]["o"], np.float32)  # [8, 22*128, 129]
        for s in range(8):
            h, w = SLOT_HEAD[qh][s]
            rows = _chunk_rows(8 * qh ^ w)
            dr2p, r4, r8 = SLOTS[s]
            sections = [(0, 0, 1024)]  # (ostage chunk, prog row0, nrows)
            if dr2p:
                sections.append((8, 0, 1024))
            if r4 is not None:
                sections.append((16, r4[0] * 128, 512))
            if r8 is not None:
                sections.append((20, r8[0] * 128, 256))
            for oc0, pr0, nr in sections:
                blk = oc[s, oc0 * 128 : oc0 * 128 + nr]
                total_sig[b, h, rows[pr0 : pr0 + nr]] += (
                    blk[:, 0:128] / blk[:, 128:129]
                )
    total = total_sig[:, :, SIG, :]
    return np.ascontiguousarray(
        total.transpose(0, 2, 1, 3).reshape(B, L, H * D)
    )


def _run(query, key, value, trace=False, **trace_kwargs):
    from concourse.bass_utils import run_bass_kernel_spmd

    nc = _get_nc()
    in_maps = _make_in_maps(query, key, value)
    res = run_bass_kernel_spmd(
        nc, in_maps, list(range(8)), trace=trace, **trace_kwargs
    )
    return _assemble(res.results), res


def kernel(query, key, value):
    # accept any array-like (np, jax, lists) and normalize to f32 numpy
    query = np.asarray(query, dtype=np.float32)
    key = np.asarray(key, dtype=np.float32)
    value = np.asarray(value, dtype=np.float32)

    # The axon-tunneled devices occasionally drop a dispatch with a
    # transient NRT_EXEC_UNIT_UNRECOVERABLE / mesh-desync error that a
    # fresh attempt survives; retry rather than failing the whole call.
    import time

    last_err = None
    for attempt in range(3):
        try:
            out, _ = _run(query, key, value)
            return out
        except Exception as e:  # noqa: BLE001 - deliberate broad retry
            last_err = e
            time.sleep(5 * (attempt + 1))
    raise last_err


# revision 61
# speedup vs baseline: 1.0274x; 1.0207x over previous
"""Dilated attention kernel for Trainium2 (8 NeuronCores, SPMD).

Problem: B=4, H=8, L=2048, D=128, dilation ratios [1,2,4,8].
Inputs  query/key/value: [32, 2048, 128] f32 (grouped (b h)).
Output: [4, 2048, 1024] f32 (b, l, h*d).

Math: for ratio dr, head h attends within the strided position subset
{p : p % dr == r}, r = h >> (3 - log2 dr); results are scatter-added over
ratios.

Key structure: permute positions by sigma(p) = rev3(p%8)*256 + p//8. Under
sigma every (dr, r) position subset becomes a CONTIGUOUS 128-row-chunk
range, and for a fixed head the dr>1 score matrices are SUBMATRICES of the
dr=1 (full, dense) score matrix. So per head we compute the dense scores
S = K^T Q and E = exp(S - 20) exactly ONCE, and every ratio's attention is
E-submatrix @ V-submatrix plus its own row-sum normalizer:

  - dr=1 uses the full E (all 16 key chunks x the core's 8 query chunks)
  - dr in {2,4,8} uses E restricted to an aligned contiguous chunk block

The PV accumulations are organized so no (l, m) product is computed twice:
each query chunk's accumulation over the 16 key chunks is split into
segments at every applicable block boundary (the block family is laminar),
and every ratio's output is a chain of vector adds over segments that
reuses smaller outputs as partial sums (see _lc_plan).

Outputs ship UNNORMALIZED with their row-sum Z as a 129th column (the ones
column of the V operand yields Z for free inside the same matmul group);
the host divides. This removes all reciprocal/normalize work on-device.

Sharding: core c = (batch b=c//2, query-half qh=c%2). SPMD: all cores run
one identical program over 8 "slots". The host maps (head, query-half) data
into slots with a per-slot XOR relabeling of 128-row chunks (XOR maps
aligned power-of-two blocks to aligned blocks), which normalizes every
core's block layout to one static slot structure:

  slot 0: dr2@[0,8) dr4@[0,4) dr8@[0,2)     slot 4: dr4@[4,8) dr8@[2,4)
  slot 1: dr2@[0,8) dr4@[0,4)               slot 5: dr4@[4,8)
  slot 2: dr2@[0,8) dr8@[4,6)               slot 6: dr8@[6,8)
  slot 3: dr2@[0,8)                         slot 7: (dr1 only)

(program chunk c holds sigma chunk c ^ mask, mask = 8*qh ^ w(qh, slot);
the program's query chunks [0,8) are the core's own query half, and every
present block's key range lies in [0,8).)

On device, per slot: S^T = K Q^T in float32r (PE pseudo-fp32, 1 cyc/row at
free >= 256), exp on ScalarE (PSUM -> bf16 P^T tiles), PV groups in bf16
with the ones column appended to V pairs host-side (129-wide contiguous
rhs keeps DMA descriptors >= 512B).
"""

import numpy as np

B, H, L, D = 4, 8, 2048, 128
DRS = [1, 2, 4, 8]
REV3 = [0, 4, 2, 6, 1, 5, 3, 7]

# sigma and its inverse as row-index arrays
P_OF_PI = np.array([(pi % 256) * 8 + REV3[pi // 256] for pi in range(L)])
SIG = np.empty(L, np.int64)
SIG[P_OF_PI] = np.arange(L)

# static slot structure: (dr2 present, dr4 chunk range, dr8 chunk range)
SLOTS = [
    (True, (0, 4), (0, 2)),
    (True, (0, 4), None),
    (True, None, (4, 6)),
    (True, None, None),
    (False, (4, 8), (2, 4)),
    (False, (4, 8), None),
    (False, None, (6, 8)),
    (False, None, None),
]
# per (qh, slot): (head, w) with chunk mask = 8*qh ^ w
SLOT_HEAD = {
    0: [(s, 0) for s in range(8)],
    1: list(zip([7, 6, 5, 4, 3, 2, 1, 0], [6, 4, 6, 0, 6, 4, 6, 0])),
}
# ostage chunk layout per slot: [0:8]=dr1, [8:16]=dr2, [16:20]=dr4, [20:22]=dr8
OS_CHUNKS = 22


def _lc_plan(s, lc):
    """PV plan for slot s, query chunk lc.

    Returns (segments, outputs): segments is a list of (m0, m1) PSUM
    accumulation groups partitioning [0, 16) at every applicable block
    boundary; outputs maps ostage chunk -> list of segment indices to sum
    (every m-chunk is matmul'd exactly once; combining is vector work).
    """
    dr2p, r4, r8 = SLOTS[s]
    bounds = {0, 8, 16} if dr2p else {0, 16}
    if r4 is not None and r4[0] <= lc < r4[1]:
        bounds |= set(r4)
    if r8 is not None and r8[0] <= lc < r8[1]:
        bounds |= set(r8)
    cuts = sorted(bounds)
    segments = list(zip(cuts[:-1], cuts[1:]))

    def covering(a, b):
        return [i for i, (x, y) in enumerate(segments) if a <= x and y <= b]

    outputs = {lc: covering(0, 16)}                       # dr1
    if dr2p:
        outputs[8 + lc] = covering(0, 8)                  # dr2
    if r4 is not None and r4[0] <= lc < r4[1]:
        outputs[16 + (lc - r4[0])] = covering(*r4)        # dr4
    if r8 is not None and r8[0] <= lc < r8[1]:
        outputs[20 + (lc - r8[0])] = covering(*r8)        # dr8
    return segments, outputs


_CACHE = {}

CFG = {
    "lookahead": 2,   # S-phases emitted ahead of each PV
    "pt_bufs": 3,
    "ps_s_bufs": 4,
    "ps_o_bufs": 4,
    "work_bufs": 2,
    "store_eng": "sync",
}

# exp engine split: ACT does chunks [0,10) exactly; DVE handles [10,16)
# with a Schraudolph-style exp approximation. Q is pre-scaled by A7 =
# 128/ln2 on the host so the matmul emits A7*s directly; then
# i16 = max(A7*s + B7, 0) truncated to int16, bits viewed as bf16, is
# exp(s-20) with ~2-3% relative error (exactly +0.0 on underflow via the
# max). One fused DVE tensor_scalar per chunk group. The approximated E
# columns only feed the dr=1 output term (dr>1 blocks all live in chunks
# [0,8)); measured end-to-end absmax rel err 5.4e-3. The exact-exp ACT
# path compensates the A7 scaling with the activation's scale parameter.
EXP_A7 = 128.0 / np.log(2.0)
EXP_B7 = 127.0 * 128.0 - 0.05798 * 128.0 + 0.5 - 20.0 * EXP_A7
# S-phase consumer groups: (kind, chunk0, n); kind A=ACT exact exp,
# D=DVE approx. Order feeds both consumer engines early.
S_GROUPS = [
    ("A", 0, 1), ("A", 1, 1), ("D", 10, 1), ("A", 2, 1), ("D", 11, 1),
    ("A", 3, 1), ("D", 12, 1), ("A", 4, 1), ("D", 13, 1), ("A", 5, 1),
    ("D", 14, 1), ("A", 6, 1), ("D", 15, 1), ("A", 7, 1), ("A", 8, 1),
    ("A", 9, 1),
]


def _build():
    """Build + compile the SPMD Bass program (identical on all 8 cores)."""
    import concourse.bass as bass  # noqa: F401
    import concourse.mybir as mybir
    import concourse.tile as tile
    from concourse import bacc

    f32 = mybir.dt.float32
    f32r = mybir.dt.float32r
    bf16 = mybir.dt.bfloat16

    nc = bacc.Bacc()
    qt = nc.dram_tensor("qt", [8, D, 1024], f32r, kind="ExternalInput")
    kt = nc.dram_tensor("kt", [8, D, L], f32r, kind="ExternalInput")
    v2 = nc.dram_tensor("v2", [8, 128, 2064], bf16, kind="ExternalInput")
    o = nc.dram_tensor("o", [8, OS_CHUNKS * 128, 129], f32, kind="ExternalOutput")

    with tile.TileContext(nc) as tc:
        with (
            tc.tile_pool(name="singles", bufs=1) as singles,
            tc.tile_pool(name="work", bufs=CFG["work_bufs"]) as work,
            tc.tile_pool(name="pt_pool", bufs=CFG["pt_bufs"]) as pt_pool,
            tc.tile_pool(name="ps_s", bufs=CFG["ps_s_bufs"], space="PSUM") as ps_s,
            tc.tile_pool(name="ps_o", bufs=CFG["ps_o_bufs"], space="PSUM") as ps_o,
        ):
            # constant bias for exp(s - 20): keeps exp values in range without
            # a data-dependent row max (|s| <= ~70)
            exp_bias = singles.tile([128, 1], f32)
            nc.vector.memset(exp_bias, -20.0)

            # PE p-state warmup: the tensor engine reaches full clock only
            # after ~3us of continuous execution. The first real matmul waits
            # ~4.3us for the first DMAs, so burn that window with dummy
            # matmuls on a zeroed tile; the ramp then completes in the DMA
            # shadow and real matmuls start at full speed.
            warm = singles.tile([128, 512], bf16, name="warm")
            nc.vector.memset(warm, 0.0)
            for _ in range(CFG.get("warmup", 6)):
                psW = ps_s.tile([128, 1, 512], f32, tag="psS", name="psW")
                nc.tensor.matmul(
                    psW[:, 0, :],
                    lhsT=warm[:, 0:128],
                    rhs=warm,
                    start=True,
                    stop=True,
                )

            head_loads = []
            all_tasks = []
            for s in range(8):
                KT = work.tile([128, 16, 128], f32r, tag="KT")
                QT = work.tile([128, 8, 128], f32r, tag="QT")
                V2 = work.tile([128, 8, 258], bf16, tag="V2")
                ostage = work.tile([128, OS_CHUNKS, 129], f32, tag="ostage")

                def load(s=s, KT=KT, QT=QT, V2=V2):
                    def dk(a, b, eng=nc.sync):
                        eng.dma_start(
                            out=KT[:, a:b, :].rearrange("d c l -> d (c l)"),
                            in_=kt[s, :, a * 128 : b * 128],
                        )

                    def dq(a, b):
                        nc.sync.dma_start(
                            out=QT[:, a:b, :].rearrange("d c l -> d (c l)"),
                            in_=qt[s, :, a * 128 : b * 128],
                        )

                    # kt pieces in S_GROUPS consumption order: chunks
                    # [0,1] [10,11] [2,3] [12,13] [4,5] [14,15] [6,7] [8,9]
                    dq(0, 4)
                    dk(0, 2)
                    dk(2, 4)
                    dk(10, 14)
                    dk(4, 8)
                    dk(14, 16)
                    dk(8, 10)
                    dq(4, 8)
                    nc.sync.dma_start(
                        out=V2.rearrange("p c x -> p (c x)"), in_=v2[s]
                    )

                head_loads.append(load)

                def make_task(s, strip, PT, KT=KT, QT=QT, V2=V2, ostage=ostage):
                    def s_phase():
                        # S^T chunk matmuls (A7-prescaled q) + exp, 512 q
                        # columns. ACT groups: exact exp with scale=1/A7.
                        # DVE groups: fused (x + B7) max 0 -> int16 viewed
                        # as bf16 (Schraudolph).
                        for kind, mc0, npair in S_GROUPS:
                            psS = ps_s.tile([128, 1, 512], f32, tag="psS", name="psS")
                            for i in range(npair):
                                nc.tensor.matmul(
                                    psS[:, i, :],
                                    lhsT=KT[:, mc0 + i, :],
                                    rhs=QT[:, strip * 4 : strip * 4 + 4, :],
                                    start=True,
                                    stop=True,
                                )
                            if kind == "A":
                                nc.scalar.activation(
                                    out=PT[:, mc0 : mc0 + npair, :],
                                    in_=psS[:, 0:npair, :],
                                    func=mybir.ActivationFunctionType.Exp,
                                    bias=exp_bias,
                                    scale=1.0 / EXP_A7,
                                )
                            else:
                                nc.vector.tensor_scalar(
                                    out=PT[
                                        :, mc0 : mc0 + npair, :
                                    ].bitcast(mybir.dt.int16),
                                    in0=psS[:, 0:npair, :],
                                    scalar1=EXP_B7,
                                    scalar2=0.0,
                                    op0=mybir.AluOpType.add,
                                    op1=mybir.AluOpType.max,
                                )
                            yield

                    def pv_phase():
                        for lcl in range(4):
                            lc = strip * 4 + lcl
                            segments, outputs = _lc_plan(s, lc)
                            tiles = [
                                ps_o.tile([128, 3, 129], f32, tag="psO", name="psO")
                                for _ in range((len(segments) + 2) // 3)
                            ]
                            aps = [
                                tiles[g // 3][:, g % 3, :]
                                for g in range(len(segments))
                            ]
                            for g, (m0, m1) in enumerate(segments):
                                for mc in range(m0, m1):
                                    nc.tensor.matmul(
                                        aps[g],
                                        lhsT=PT[:, mc, lcl * 128 : (lcl + 1) * 128],
                                        rhs=V2[
                                            :,
                                            mc // 2,
                                            (mc % 2) * 129 : (mc % 2) * 129 + 129,
                                        ],
                                        start=(mc == m0),
                                        stop=(mc == m1 - 1),
                                    )
                            # combine segments into staged outputs, reusing
                            # smaller outputs as partial sums (blocks are
                            # laminar). Outputs processed smallest-first.
                            done = {}  # (m0, m1) range -> ostage chunk
                            for oc in sorted(
                                outputs, key=lambda c: len(outputs[c])
                            ):
                                segs = outputs[oc]
                                lo_, hi_ = (
                                    segments[segs[0]][0],
                                    segments[segs[-1]][1],
                                )
                                # greedy cover of [lo_, hi_): prefer computed
                                # sub-outputs, else raw segments
                                terms = []
                                pos = lo_
                                while pos < hi_:
                                    best = None
                                    for (a, b), c in done.items():
                                        if a == pos and b <= hi_ and (
                                            best is None or b > best[0]
                                        ):
                                            best = (b, ("chunk", c))
                                    if best is None:
                                        gi = next(
                                            i
                                            for i, (a, b) in enumerate(segments)
                                            if a == pos
                                        )
                                        best = (
                                            segments[gi][1],
                                            ("seg", gi),
                                        )
                                    pos = best[0]
                                    terms.append(best[1])
                                dst = ostage[:, oc, :]

                                def ap_of(term):
                                    kind, i = term
                                    return (
                                        aps[i]
                                        if kind == "seg"
                                        else ostage[:, i, :]
                                    )

                                if len(terms) == 1:
                                    nc.vector.tensor_copy(
                                        out=dst, in_=ap_of(terms[0])
                                    )
                                else:
                                    nc.vector.tensor_add(
                                        out=dst,
                                        in0=ap_of(terms[1]),
                                        in1=ap_of(terms[0]),
                                    )
                                    for term in terms[2:]:
                                        nc.vector.tensor_add(
                                            out=dst, in0=dst, in1=ap_of(term)
                                        )
                                done[(lo_, hi_)] = oc
                            yield
                        # store every ostage run whose source l-chunks are
                        # complete after this strip; the last slot splits its
                        # final run so the kernel tail ends on a small DMA
                        store_eng = getattr(nc, CFG["store_eng"])
                        dr2p, r4, r8 = SLOTS[s]
                        lo, hi = strip * 4, strip * 4 + 4
                        runs = []
                        runs.append((lo, hi))                       # dr1 part
                        if dr2p:
                            runs.append((8 + lo, 8 + hi))           # dr2 part
                        if r4 is not None and r4 == (lo, hi):
                            runs.append((16, 20))
                        if r8 is not None and lo <= r8[0] < hi:
                            runs.append((20, 22))
                        if s == 7 and strip == 1:
                            # keep the final DMA tiny: it gates kernel end
                            merged = [(4, 7), (7, 8)]
                        else:
                            runs.sort()
                            merged = [list(runs[0])]
                            for a, b in runs[1:]:
                                if a == merged[-1][1]:
                                    merged[-1][1] = b
                                else:
                                    merged.append([a, b])
                        for a, b in merged:
                            store_eng.dma_start(
                                out=o[s, a * 128 : b * 128, :].rearrange(
                                    "(c p) d -> p c d", p=128
                                ),
                                in_=ostage[:, a:b, :],
                            )

                    return s_phase, pv_phase

                for strip in range(2):
                    PT = pt_pool.tile([128, 16, 512], bf16, tag="pt", name="PT")
                    all_tasks.append(make_task(s, strip, PT))

            # software pipeline: emit S(i+LOOK) before PV(i); loads one slot
            # ahead so HWDGE ring order matches consumption order
            emitted_loads = [False] * 8

            def ensure_loads(j):
                if 0 <= j < 8 and not emitted_loads[j]:
                    emitted_loads[j] = True
                    head_loads[j]()

            def drain(gen):
                for _ in gen:
                    pass

            LOOK = CFG["lookahead"]
            NT = len(all_tasks)
            ensure_loads(0)
            ensure_loads(1)
            if CFG.get("ilv"):
                # fine-grained interleave: R S-groups emitted per PV yield,
                # S-stream runs up to LOOK tasks ahead of the PV stream
                R = CFG.get("ilv_ratio", 3)
                s_gens = [t[0]() for t in all_tasks]
                s_done = [False] * NT
                s_next = 0

                def step_s(limit, n):
                    nonlocal s_next
                    took = 0
                    while took < n and s_next <= min(limit, NT - 1):
                        if s_done[s_next]:
                            s_next += 1
                            continue
                        ensure_loads(s_next // 2 + 1)
                        try:
                            next(s_gens[s_next])
                            took += 1
                        except StopIteration:
                            s_done[s_next] = True
                            s_next += 1

                for i in range(NT):
                    step_s(i, 10**9)
                    while not s_done[i]:
                        step_s(i, 10**9)
                    for _ in all_tasks[i][1]():
                        step_s(i + LOOK, R)
            else:
                for j in range(min(LOOK, NT)):
                    drain(all_tasks[j][0]())
                for i in range(NT):
                    # PV first: its DVE combines queue ahead of the next S
                    # phase's DVE exp ops, so psO buffers recycle promptly
                    drain(all_tasks[i][1]())
                    if i + LOOK < NT:
                        ensure_loads((i + LOOK) // 2 + 1)
                        drain(all_tasks[i + LOOK][0]())

    nc.compile()
    return nc


def _get_nc():
    if "nc" not in _CACHE:
        _CACHE["nc"] = _build()
    return _CACHE["nc"]


def _chunk_rows(mask):
    c = np.arange(16) ^ mask
    return (c[:, None] * 128 + np.arange(128)[None, :]).reshape(-1)


def _make_in_maps(query, key, value):
    import ml_dtypes

    q = query.reshape(B, H, L, D)[:, :, P_OF_PI, :]
    k = key.reshape(B, H, L, D)[:, :, P_OF_PI, :]
    v = value.reshape(B, H, L, D)[:, :, P_OF_PI, :]
    in_maps = []
    for c in range(8):
        b, qh = c // 2, c % 2
        ktd = np.empty((8, D, L), np.float32)
        qtd = np.empty((8, D, 1024), np.float32)
        v2d = np.ones((8, 128, 8, 2, 129), np.float32)
        for s in range(8):
            h, w = SLOT_HEAD[qh][s]
            rows = _chunk_rows(8 * qh ^ w)
            ktd[s] = k[b, h][rows].T
            # A7 pre-scale: the S matmul emits A7*s (see EXP_A7 notes)
            qtd[s] = q[b, h][rows[:1024]].T * EXP_A7
            # V pairs: [p, t, half, 0:128] = v row 256t + 128 half + p; col 128 = 1
            v2d[s, :, :, :, 0:128] = (
                v[b, h][rows].reshape(8, 2, 128, 128).transpose(2, 0, 1, 3)
            )
        in_maps.append(
            {
                "qt": qtd,
                "kt": ktd,
                "v2": np.ascontiguousarray(
                    v2d.reshape(8, 128, 2064)
                ).astype(ml_dtypes.bfloat16),
            }
        )
    return in_maps


def _assemble(results):
    total_sig = np.zeros((B, H, L, D), np.float32)
    for c in range(8):
        b, qh = c // 2, c % 2
        oc = np.asarray(results[c]["o"], np.float32)  # [8, 22*128, 129]
        for s in range(8):
            h, w = SLOT_HEAD[qh][s]
            rows = _chunk_rows(8 * qh ^ w)
            dr2p, r4, r8 = SLOTS[s]
            sections = [(0, 0, 1024)]  # (ostage chunk, prog row0, nrows)
            if dr2p:
                sections.append((8, 0, 1024))
            if r4 is not None:
                sections.append((16, r4[0] * 128, 512))
            if r8 is not None:
                sections.append((20, r8[0] * 128, 256))
            for oc0, pr0, nr in sections:
                blk = oc[s, oc0 * 128 : oc0 * 128 + nr]
                total_sig[b, h, rows[pr0 : pr0 + nr]] += (
                    blk[:, 0:128] / blk[:, 128:129]
                )
    total = total_sig[:, :, SIG, :]
    return np.ascontiguousarray(
        total.transpose(0, 2, 1, 3).reshape(B, L, H * D)
    )


def _run(query, key, value, trace=False, **trace_kwargs):
    from concourse.bass_utils import run_bass_kernel_spmd

    nc = _get_nc()
    in_maps = _make_in_maps(query, key, value)
    res = run_bass_kernel_spmd(
        nc, in_maps, list(range(8)), trace=trace, **trace_kwargs
    )
    return _assemble(res.results), res


def kernel(query, key, value):
    # accept any array-like (np, jax, lists) and normalize to f32 numpy
    query = np.asarray(query, dtype=np.float32)
    key = np.asarray(key, dtype=np.float32)
    value = np.asarray(value, dtype=np.float32)

    # The axon-tunneled devices occasionally drop a dispatch with a
    # transient NRT_EXEC_UNIT_UNRECOVERABLE / mesh-desync error that a
    # fresh attempt survives; retry rather than failing the whole call.
    import time

    last_err = None
    for attempt in range(3):
        try:
            out, _ = _run(query, key, value)
            return out
        except Exception as e:  # noqa: BLE001 - deliberate broad retry
            last_err = e
            time.sleep(5 * (attempt + 1))
    raise last_err


# revision 63
# speedup vs baseline: 1.0315x; 1.0040x over previous
"""Dilated attention kernel for Trainium2 (8 NeuronCores, SPMD).

Problem: B=4, H=8, L=2048, D=128, dilation ratios [1,2,4,8].
Inputs  query/key/value: [32, 2048, 128] f32 (grouped (b h)).
Output: [4, 2048, 1024] f32 (b, l, h*d).

Math: for ratio dr, head h attends within the strided position subset
{p : p % dr == r}, r = h >> (3 - log2 dr); results are scatter-added over
ratios.

Key structure: permute positions by sigma(p) = rev3(p%8)*256 + p//8. Under
sigma every (dr, r) position subset becomes a CONTIGUOUS 128-row-chunk
range, and for a fixed head the dr>1 score matrices are SUBMATRICES of the
dr=1 (full, dense) score matrix. So per head we compute the dense scores
S = K^T Q and E = exp(S - 20) exactly ONCE, and every ratio's attention is
E-submatrix @ V-submatrix plus its own row-sum normalizer:

  - dr=1 uses the full E (all 16 key chunks x the core's 8 query chunks)
  - dr in {2,4,8} uses E restricted to an aligned contiguous chunk block

The PV accumulations are organized so no (l, m) product is computed twice:
each query chunk's accumulation over the 16 key chunks is split into
segments at every applicable block boundary (the block family is laminar),
and every ratio's output is a chain of vector adds over segments that
reuses smaller outputs as partial sums (see _lc_plan).

Outputs ship UNNORMALIZED with their row-sum Z as a 129th column (the ones
column of the V operand yields Z for free inside the same matmul group);
the host divides. This removes all reciprocal/normalize work on-device.

Sharding: core c = (batch b=c//2, query-half qh=c%2). SPMD: all cores run
one identical program over 8 "slots". The host maps (head, query-half) data
into slots with a per-slot XOR relabeling of 128-row chunks (XOR maps
aligned power-of-two blocks to aligned blocks), which normalizes every
core's block layout to one static slot structure:

  slot 0: dr2@[0,8) dr4@[0,4) dr8@[0,2)     slot 4: dr4@[4,8) dr8@[2,4)
  slot 1: dr2@[0,8) dr4@[0,4)               slot 5: dr4@[4,8)
  slot 2: dr2@[0,8) dr8@[4,6)               slot 6: dr8@[6,8)
  slot 3: dr2@[0,8)                         slot 7: (dr1 only)

(program chunk c holds sigma chunk c ^ mask, mask = 8*qh ^ w(qh, slot);
the program's query chunks [0,8) are the core's own query half, and every
present block's key range lies in [0,8).)

On device, per slot: S^T = K Q^T in float32r (PE pseudo-fp32, 1 cyc/row at
free >= 256), exp on ScalarE (PSUM -> bf16 P^T tiles), PV groups in bf16
with the ones column appended to V pairs host-side (129-wide contiguous
rhs keeps DMA descriptors >= 512B).
"""

import numpy as np

B, H, L, D = 4, 8, 2048, 128
DRS = [1, 2, 4, 8]
REV3 = [0, 4, 2, 6, 1, 5, 3, 7]

# sigma and its inverse as row-index arrays
P_OF_PI = np.array([(pi % 256) * 8 + REV3[pi // 256] for pi in range(L)])
SIG = np.empty(L, np.int64)
SIG[P_OF_PI] = np.arange(L)

# static slot structure: (dr2 present, dr4 chunk range, dr8 chunk range)
SLOTS = [
    (True, (0, 4), (0, 2)),
    (True, (0, 4), None),
    (True, None, (4, 6)),
    (True, None, None),
    (False, (4, 8), (2, 4)),
    (False, (4, 8), None),
    (False, None, (6, 8)),
    (False, None, None),
]
# per (qh, slot): (head, w) with chunk mask = 8*qh ^ w
SLOT_HEAD = {
    0: [(s, 0) for s in range(8)],
    1: list(zip([7, 6, 5, 4, 3, 2, 1, 0], [6, 4, 6, 0, 6, 4, 6, 0])),
}
# ostage chunk layout per slot: [0:8]=dr1, [8:16]=dr2, [16:20]=dr4, [20:22]=dr8
OS_CHUNKS = 22


def _lc_plan(s, lc):
    """PV plan for slot s, query chunk lc.

    Returns (segments, outputs): segments is a list of (m0, m1) PSUM
    accumulation groups partitioning [0, 16) at every applicable block
    boundary; outputs maps ostage chunk -> list of segment indices to sum
    (every m-chunk is matmul'd exactly once; combining is vector work).
    """
    dr2p, r4, r8 = SLOTS[s]
    bounds = {0, 8, 16} if dr2p else {0, 16}
    if r4 is not None and r4[0] <= lc < r4[1]:
        bounds |= set(r4)
    if r8 is not None and r8[0] <= lc < r8[1]:
        bounds |= set(r8)
    cuts = sorted(bounds)
    segments = list(zip(cuts[:-1], cuts[1:]))

    def covering(a, b):
        return [i for i, (x, y) in enumerate(segments) if a <= x and y <= b]

    outputs = {lc: covering(0, 16)}                       # dr1
    if dr2p:
        outputs[8 + lc] = covering(0, 8)                  # dr2
    if r4 is not None and r4[0] <= lc < r4[1]:
        outputs[16 + (lc - r4[0])] = covering(*r4)        # dr4
    if r8 is not None and r8[0] <= lc < r8[1]:
        outputs[20 + (lc - r8[0])] = covering(*r8)        # dr8
    return segments, outputs


_CACHE = {}

CFG = {
    "lookahead": 3,   # S-phases emitted ahead of each PV
    "pt_bufs": 4,
    "ps_s_bufs": 4,
    "ps_o_bufs": 4,
    "work_bufs": 2,
    "store_eng": "sync",
}

# exp engine split: ACT does chunks [0,10) exactly; DVE handles [10,16)
# with a Schraudolph-style exp approximation. Q is pre-scaled by A7 =
# 128/ln2 on the host so the matmul emits A7*s directly; then
# i16 = max(A7*s + B7, 0) truncated to int16, bits viewed as bf16, is
# exp(s-20) with ~2-3% relative error (exactly +0.0 on underflow via the
# max). One fused DVE tensor_scalar per chunk group. The approximated E
# columns only feed the dr=1 output term (dr>1 blocks all live in chunks
# [0,8)); measured end-to-end absmax rel err 5.4e-3. The exact-exp ACT
# path compensates the A7 scaling with the activation's scale parameter.
EXP_A7 = 128.0 / np.log(2.0)
EXP_B7 = 127.0 * 128.0 - 0.05798 * 128.0 + 0.5 - 20.0 * EXP_A7
# S-phase consumer groups: (kind, chunk0, n); kind A=ACT exact exp,
# D=DVE approx. Order feeds both consumer engines early.
S_GROUPS = [
    ("A", 0, 1), ("A", 1, 1), ("D", 10, 1), ("A", 2, 1), ("D", 11, 1),
    ("A", 3, 1), ("D", 12, 1), ("A", 4, 1), ("D", 13, 1), ("A", 5, 1),
    ("D", 14, 1), ("A", 6, 1), ("D", 15, 1), ("A", 7, 1), ("A", 8, 1),
    ("A", 9, 1),
]


def _build():
    """Build + compile the SPMD Bass program (identical on all 8 cores)."""
    import concourse.bass as bass  # noqa: F401
    import concourse.mybir as mybir
    import concourse.tile as tile
    from concourse import bacc

    f32 = mybir.dt.float32
    f32r = mybir.dt.float32r
    bf16 = mybir.dt.bfloat16

    nc = bacc.Bacc()
    qt = nc.dram_tensor("qt", [8, D, 1024], f32r, kind="ExternalInput")
    kt = nc.dram_tensor("kt", [8, D, L], f32r, kind="ExternalInput")
    v2 = nc.dram_tensor("v2", [8, 128, 2064], bf16, kind="ExternalInput")
    o = nc.dram_tensor("o", [8, OS_CHUNKS * 128, 129], f32, kind="ExternalOutput")

    with tile.TileContext(nc) as tc:
        with (
            tc.tile_pool(name="singles", bufs=1) as singles,
            tc.tile_pool(name="work", bufs=CFG["work_bufs"]) as work,
            tc.tile_pool(name="pt_pool", bufs=CFG["pt_bufs"]) as pt_pool,
            tc.tile_pool(name="ps_s", bufs=CFG["ps_s_bufs"], space="PSUM") as ps_s,
            tc.tile_pool(name="ps_o", bufs=CFG["ps_o_bufs"], space="PSUM") as ps_o,
        ):
            # constant bias for exp(s - 20): keeps exp values in range without
            # a data-dependent row max (|s| <= ~70)
            exp_bias = singles.tile([128, 1], f32)
            nc.vector.memset(exp_bias, -20.0)

            # PE p-state warmup: the tensor engine reaches full clock only
            # after ~3us of continuous execution. The first real matmul waits
            # ~4.3us for the first DMAs, so burn that window with dummy
            # matmuls on a zeroed tile; the ramp then completes in the DMA
            # shadow and real matmuls start at full speed.
            warm = singles.tile([128, 512], bf16, name="warm")
            nc.vector.memset(warm, 0.0)
            for _ in range(CFG.get("warmup", 6)):
                psW = ps_s.tile([128, 1, 512], f32, tag="psS", name="psW")
                nc.tensor.matmul(
                    psW[:, 0, :],
                    lhsT=warm[:, 0:128],
                    rhs=warm,
                    start=True,
                    stop=True,
                )

            head_loads = []
            all_tasks = []
            for s in range(8):
                KT = work.tile([128, 16, 128], f32r, tag="KT")
                QT = work.tile([128, 8, 128], f32r, tag="QT")
                V2 = work.tile([128, 8, 258], bf16, tag="V2")
                ostage = work.tile([128, OS_CHUNKS, 129], f32, tag="ostage")

                def load(s=s, KT=KT, QT=QT, V2=V2):
                    def dk(a, b, eng=nc.sync):
                        eng.dma_start(
                            out=KT[:, a:b, :].rearrange("d c l -> d (c l)"),
                            in_=kt[s, :, a * 128 : b * 128],
                        )

                    def dq(a, b):
                        nc.sync.dma_start(
                            out=QT[:, a:b, :].rearrange("d c l -> d (c l)"),
                            in_=qt[s, :, a * 128 : b * 128],
                        )

                    # kt pieces in S_GROUPS consumption order: chunks
                    # [0,1] [10,11] [2,3] [12,13] [4,5] [14,15] [6,7] [8,9]
                    dq(0, 4)
                    dk(0, 2)
                    dk(2, 4)
                    dk(10, 14)
                    dk(4, 8)
                    dk(14, 16)
                    dk(8, 10)
                    dq(4, 8)
                    nc.sync.dma_start(
                        out=V2.rearrange("p c x -> p (c x)"), in_=v2[s]
                    )

                head_loads.append(load)

                def make_task(s, strip, PT, KT=KT, QT=QT, V2=V2, ostage=ostage):
                    def s_phase():
                        # S^T chunk matmuls (A7-prescaled q) + exp, 512 q
                        # columns. ACT groups: exact exp with scale=1/A7.
                        # DVE groups: fused (x + B7) max 0 -> int16 viewed
                        # as bf16 (Schraudolph).
                        for kind, mc0, npair in S_GROUPS:
                            psS = ps_s.tile([128, 1, 512], f32, tag="psS", name="psS")
                            for i in range(npair):
                                nc.tensor.matmul(
                                    psS[:, i, :],
                                    lhsT=KT[:, mc0 + i, :],
                                    rhs=QT[:, strip * 4 : strip * 4 + 4, :],
                                    start=True,
                                    stop=True,
                                )
                            if kind == "A":
                                nc.scalar.activation(
                                    out=PT[:, mc0 : mc0 + npair, :],
                                    in_=psS[:, 0:npair, :],
                                    func=mybir.ActivationFunctionType.Exp,
                                    bias=exp_bias,
                                    scale=1.0 / EXP_A7,
                                )
                            else:
                                nc.vector.tensor_scalar(
                                    out=PT[
                                        :, mc0 : mc0 + npair, :
                                    ].bitcast(mybir.dt.int16),
                                    in0=psS[:, 0:npair, :],
                                    scalar1=EXP_B7,
                                    scalar2=0.0,
                                    op0=mybir.AluOpType.add,
                                    op1=mybir.AluOpType.max,
                                )
                            yield

                    def pv_phase():
                        for lcl in range(4):
                            lc = strip * 4 + lcl
                            segments, outputs = _lc_plan(s, lc)
                            tiles = [
                                ps_o.tile([128, 3, 129], f32, tag="psO", name="psO")
                                for _ in range((len(segments) + 2) // 3)
                            ]
                            aps = [
                                tiles[g // 3][:, g % 3, :]
                                for g in range(len(segments))
                            ]
                            for g, (m0, m1) in enumerate(segments):
                                for mc in range(m0, m1):
                                    nc.tensor.matmul(
                                        aps[g],
                                        lhsT=PT[:, mc, lcl * 128 : (lcl + 1) * 128],
                                        rhs=V2[
                                            :,
                                            mc // 2,
                                            (mc % 2) * 129 : (mc % 2) * 129 + 129,
                                        ],
                                        start=(mc == m0),
                                        stop=(mc == m1 - 1),
                                    )
                            # combine segments into staged outputs, reusing
                            # smaller outputs as partial sums (blocks are
                            # laminar). Outputs processed smallest-first.
                            done = {}  # (m0, m1) range -> ostage chunk
                            for oc in sorted(
                                outputs, key=lambda c: len(outputs[c])
                            ):
                                segs = outputs[oc]
                                lo_, hi_ = (
                                    segments[segs[0]][0],
                                    segments[segs[-1]][1],
                                )
                                # greedy cover of [lo_, hi_): prefer computed
                                # sub-outputs, else raw segments
                                terms = []
                                pos = lo_
                                while pos < hi_:
                                    best = None
                                    for (a, b), c in done.items():
                                        if a == pos and b <= hi_ and (
                                            best is None or b > best[0]
                                        ):
                                            best = (b, ("chunk", c))
                                    if best is None:
                                        gi = next(
                                            i
                                            for i, (a, b) in enumerate(segments)
                                            if a == pos
                                        )
                                        best = (
                                            segments[gi][1],
                                            ("seg", gi),
                                        )
                                    pos = best[0]
                                    terms.append(best[1])
                                dst = ostage[:, oc, :]

                                def ap_of(term):
                                    kind, i = term
                                    return (
                                        aps[i]
                                        if kind == "seg"
                                        else ostage[:, i, :]
                                    )

                                if len(terms) == 1:
                                    nc.vector.tensor_copy(
                                        out=dst, in_=ap_of(terms[0])
                                    )
                                else:
                                    nc.vector.tensor_add(
                                        out=dst,
                                        in0=ap_of(terms[1]),
                                        in1=ap_of(terms[0]),
                                    )
                                    for term in terms[2:]:
                                        nc.vector.tensor_add(
                                            out=dst, in0=dst, in1=ap_of(term)
                                        )
                                done[(lo_, hi_)] = oc
                            yield
                        # store every ostage run whose source l-chunks are
                        # complete after this strip; the last slot splits its
                        # final run so the kernel tail ends on a small DMA
                        store_eng = getattr(nc, CFG["store_eng"])
                        dr2p, r4, r8 = SLOTS[s]
                        lo, hi = strip * 4, strip * 4 + 4
                        runs = []
                        runs.append((lo, hi))                       # dr1 part
                        if dr2p:
                            runs.append((8 + lo, 8 + hi))           # dr2 part
                        if r4 is not None and r4 == (lo, hi):
                            runs.append((16, 20))
                        if r8 is not None and lo <= r8[0] < hi:
                            runs.append((20, 22))
                        if s == 7 and strip == 1:
                            # keep the final DMA tiny: it gates kernel end
                            merged = [(4, 7), (7, 8)]
                        else:
                            runs.sort()
                            merged = [list(runs[0])]
                            for a, b in runs[1:]:
                                if a == merged[-1][1]:
                                    merged[-1][1] = b
                                else:
                                    merged.append([a, b])
                        for a, b in merged:
                            store_eng.dma_start(
                                out=o[s, a * 128 : b * 128, :].rearrange(
                                    "(c p) d -> p c d", p=128
                                ),
                                in_=ostage[:, a:b, :],
                            )

                    return s_phase, pv_phase

                for strip in range(2):
                    PT = pt_pool.tile([128, 16, 512], bf16, tag="pt", name="PT")
                    all_tasks.append(make_task(s, strip, PT))

            # software pipeline: emit S(i+LOOK) before PV(i); loads one slot
            # ahead so HWDGE ring order matches consumption order
            emitted_loads = [False] * 8

            def ensure_loads(j):
                if 0 <= j < 8 and not emitted_loads[j]:
                    emitted_loads[j] = True
                    head_loads[j]()

            def drain(gen):
                for _ in gen:
                    pass

            LOOK = CFG["lookahead"]
            NT = len(all_tasks)
            ensure_loads(0)
            ensure_loads(1)
            if CFG.get("ilv"):
                # fine-grained interleave: R S-groups emitted per PV yield,
                # S-stream runs up to LOOK tasks ahead of the PV stream
                R = CFG.get("ilv_ratio", 3)
                s_gens = [t[0]() for t in all_tasks]
                s_done = [False] * NT
                s_next = 0

                def step_s(limit, n):
                    nonlocal s_next
                    took = 0
                    while took < n and s_next <= min(limit, NT - 1):
                        if s_done[s_next]:
                            s_next += 1
                            continue
                        ensure_loads(s_next // 2 + 1)
                        try:
                            next(s_gens[s_next])
                            took += 1
                        except StopIteration:
                            s_done[s_next] = True
                            s_next += 1

                for i in range(NT):
                    step_s(i, 10**9)
                    while not s_done[i]:
                        step_s(i, 10**9)
                    for _ in all_tasks[i][1]():
                        step_s(i + LOOK, R)
            else:
                for j in range(min(LOOK, NT)):
                    drain(all_tasks[j][0]())
                for i in range(NT):
                    # PV first: its DVE combines queue ahead of the next S
                    # phase's DVE exp ops, so psO buffers recycle promptly
                    drain(all_tasks[i][1]())
                    if i + LOOK < NT:
                        ensure_loads((i + LOOK) // 2 + 1)
                        drain(all_tasks[i + LOOK][0]())

    nc.compile()
    return nc


def _get_nc():
    if "nc" not in _CACHE:
        _CACHE["nc"] = _build()
    return _CACHE["nc"]


def _chunk_rows(mask):
    c = np.arange(16) ^ mask
    return (c[:, None] * 128 + np.arange(128)[None, :]).reshape(-1)


def _make_in_maps(query, key, value):
    import ml_dtypes

    q = query.reshape(B, H, L, D)[:, :, P_OF_PI, :]
    k = key.reshape(B, H, L, D)[:, :, P_OF_PI, :]
    v = value.reshape(B, H, L, D)[:, :, P_OF_PI, :]
    in_maps = []
    for c in range(8):
        b, qh = c // 2, c % 2
        ktd = np.empty((8, D, L), np.float32)
        qtd = np.empty((8, D, 1024), np.float32)
        v2d = np.ones((8, 128, 8, 2, 129), np.float32)
        for s in range(8):
            h, w = SLOT_HEAD[qh][s]
            rows = _chunk_rows(8 * qh ^ w)
            ktd[s] = k[b, h][rows].T
            # A7 pre-scale: the S matmul emits A7*s (see EXP_A7 notes)
            qtd[s] = q[b, h][rows[:1024]].T * EXP_A7
            # V pairs: [p, t, half, 0:128] = v row 256t + 128 half + p; col 128 = 1
            v2d[s, :, :, :, 0:128] = (
                v[b, h][rows].reshape(8, 2, 128, 128).transpose(2, 0, 1, 3)
            )
        in_maps.append(
            {
                "qt": qtd,
                "kt": ktd,
                "v2": np.ascontiguousarray(
                    v2d.reshape(8, 128, 2064)
                ).astype(ml_dtypes.bfloat16),
            }
        )
    return in_maps


def _assemble(results):
    total_sig = np.zeros((B, H, L, D), np.float32)
    for c in range(8):
        b, qh = c // 2, c % 2
        oc = np.asarray(results[c]["o"], np.float32)  # [8, 22*128, 129]
        for s in range(8):
            h, w = SLOT_HEAD[qh][s]
            rows = _chunk_rows(8 * qh ^ w)
            dr2p, r4, r8 = SLOTS[s]
            sections = [(0, 0, 1024)]  # (ostage chunk, prog row0, nrows)
            if dr2p:
                sections.append((8, 0, 1024))
            if r4 is not None:
                sections.append((16, r4[0] * 128, 512))
            if r8 is not None:
                sections.append((20, r8[0] * 128, 256))
            for oc0, pr0, nr in sections:
                blk = oc[s, oc0 * 128 : oc0 * 128 + nr]
                total_sig[b, h, rows[pr0 : pr0 + nr]] += (
                    blk[:, 0:128] / blk[:, 128:129]
                )
    total = total_sig[:, :, SIG, :]
    return np.ascontiguousarray(
        total.transpose(0, 2, 1, 3).reshape(B, L, H * D)
    )


def _run(query, key, value, trace=False, **trace_kwargs):
    from concourse.bass_utils import run_bass_kernel_spmd

    nc = _get_nc()
    in_maps = _make_in_maps(query, key, value)
    res = run_bass_kernel_spmd(
        nc, in_maps, list(range(8)), trace=trace, **trace_kwargs
    )
    return _assemble(res.results), res


def kernel(query, key, value):
    # accept any array-like (np, jax, lists) and normalize to f32 numpy
    query = np.asarray(query, dtype=np.float32)
    key = np.asarray(key, dtype=np.float32)
    value = np.asarray(value, dtype=np.float32)

    # The axon-tunneled devices occasionally drop a dispatch with a
    # transient NRT_EXEC_UNIT_UNRECOVERABLE / mesh-desync error that a
    # fresh attempt survives; retry rather than failing the whole call.
    import time

    last_err = None
    for attempt in range(3):
        try:
            out, _ = _run(query, key, value)
            return out
        except Exception as e:  # noqa: BLE001 - deliberate broad retry
            last_err = e
            time.sleep(5 * (attempt + 1))
    raise last_err
